# revision 2
# baseline (speedup 1.0000x reference)
"""Triangular matmul C = triu(triu(A) @ triu(B)) on 8 TRN2 NeuronCores.

Structure: the (I, K, J) block-tetrahedron {I <= K <= J} (128x128 blocks,
N=4096 -> 32 blocks/side) is sharded by output row-block I across the 8
cores with a work-balanced assignment.  Each core runs its own statically
addressed program inside a `tc.If(partition_id == c)` block.

Per core: row-blocks are processed in groups of <=4 that share one sweep
over the B strips (B[K, K*128:] for K >= min(group)).  The J axis is cut
into 512-wide phases; each group member I gets one PSUM bank per phase
(double-buffered), accumulating A^T[K,I] @ B[K, phase-window] over K, then
evicting to the output.

Numerics: fp32 operands are split on the host into bf16 (hi, lo) pairs and
each block product uses 3 bf16 matmuls (Ah@Bh + Ah@Bl + Al@Bh), giving
~5e-6 relative error vs the fp32 reference at 3/4 the cost of the PE's
native 4-pass fp32 mode (measured on HW: rel_absmax 4.8e-6 at N=4096).

DMA discipline: one ~256KB DMA per (K-strip, phase) carrying hi and lo
planes together (Bcat = [Bh | Bl]), issued alternately from the SP and DVE
sequencers; A^T strips are host-packed per core (apack) so each strip is a
few large-line DMAs on the GpSimd sequencer; PSUM evictions copy on ACT and
store from its sequencer.  This keeps ~16 DMA engines busy without
sequencer issue serialization.

The kernel takes FULL (unsharded) inputs and returns the FULL output.
"""

import numpy as np

N = 4096
BLK = 128
NB = N // BLK  # 32
N_CORES = 8
PHASE = 512  # J-phase width (one PSUM bank of fp32)
MODE = "bf16x3"  # "bf16x3" | "fp32r" | "fp32"

# Work-balanced assignment of row-blocks I to cores (work(I) = T(32-I),
# T(m)=m(m+1)/2; bins balanced to 743..752 of 5984/8=748).
BINS = [
    [0, 14, 23],
    [1, 15, 21, 25, 29],
    [2, 13, 20, 28],
    [4, 12, 16],
    [3, 10, 22],
    [6, 9, 17, 30],
    [5, 11, 19, 24, 27, 31],
    [7, 8, 18, 26],
]
MAXB = max(len(b) for b in BINS)  # output row-slots per core
# A-pack slot layout: per core, the A^T strips (one 128x128 block per slot,
# hi+lo planes) for each owned I, K = I..31, concatenated.
ABASE = [
    {I: int(np.cumsum([0] + [NB - J for J in sorted(b)])[i]) for i, I in enumerate(sorted(b))}
    for b in BINS
]
NSLOT = 80  # >= max per-core total blocks (75)
ACHUNK = 4  # A-load DMA granularity in k-blocks


def _groups(bin_is):
    """Split a sorted bin into contiguous groups of <=4 minimizing the
    total B-strip traffic sum(T(32 - min(group)))."""
    Is = sorted(bin_is)
    t = lambda m: m * (m + 1) // 2
    best = None

    def rec(i, acc, parts):
        nonlocal best
        if i == len(Is):
            if best is None or acc < best[0]:
                best = (acc, [list(p) for p in parts])
            return
        for g in range(1, 5):
            if i + g <= len(Is):
                rec(i + g, acc + t(NB - Is[i]), parts + [Is[i : i + g]])

    rec(0, 0, [])
    return best[1]


def _emit_core(nc, tc, pools, dram_io, core, mode, variant="full"):
    """K-major schedule: one row-block I at a time, full output row in PSUM
    (8 banks), K-sweep with each A-tile's weights amortized over all J-chunks
    (weight switches are ~180ns on HW; this gives 2 per (I,K) instead of 2
    per (I,K,phase))."""
    apool, bpool, cpool, psum_pool = pools
    import concourse.mybir as mybir

    f32 = mybir.dt.float32
    nplane = 2 if mode == "bf16x3" else 1
    dt_in = {
        "bf16x3": mybir.dt.bfloat16,
        "fp32r": mybir.dt.float32r,
        "fp32": f32,
    }[mode]
    apack, bcat, cpart = dram_io["apack"], dram_io["bcat"], dram_io["cpart"]
    bcat3 = bcat.rearrange("k (t n) -> k t n", t=nplane)

    bin_is = BINS[core]
    slot = {I: s for s, I in enumerate(sorted(bin_is))}
    bdma_engines = [nc.sync, nc.scalar]
    bdma_i = 0

    static_b = None
    if "nobdma" in variant:
        static_b = []
        for ci in range(NB // 8):
            sb_t = bpool.tile(
                [BLK, nplane, 2 * PHASE], dt_in, name=f"sb_{ci}", tag=f"sb{ci}", bufs=1
            )
            nc.gpsimd.memset(sb_t[:], 0.5)
            static_b.append(sb_t)

    for I in sorted(bin_is):
        nblk = NB - I
        base = ABASE[core][I]
        a_t = apool.tile([BLK, nblk, nplane, BLK], dt_in, name=f"a_{I}", tag="a")
        for j0 in range(0, nblk, ACHUNK):
            j1 = min(j0 + ACHUNK, nblk)
            nc.gpsimd.dma_start(
                a_t[:, j0:j1, :, :], apack[:, base + j0 : base + j1, :, :]
            )
        c0 = I // 4  # first active PSUM bank / J-chunk
        ps = {
            c: psum_pool.tile([BLK, PHASE], f32, name=f"ps_{I}_{c}", tag=f"ps{c}")
            for c in range(c0, NB // 4)
        }

        for K in range(I, NB):
            kb = K - I
            # B strip double-chunks (1024 cols -> 2KB DMA lines; hi+lo planes
            # in one DMA).  Each plane feeds two 512-wide matmuls (PSUM bank
            # limit).
            b_ts = {}
            for d in range(K // 8, NB // 8):
                pstart = max(K * BLK, 2 * PHASE * d)
                width = 2 * PHASE * (d + 1) - pstart
                if "nobdma" in variant:
                    b_ts[d] = (static_b[d], pstart, width)
                    continue
                b_t = bpool.tile(
                    [BLK, nplane, 2 * PHASE], dt_in, name=f"b_{K}_{d}", tag="b"
                )
                for t in range(nplane):
                    eng = bdma_engines[bdma_i % len(bdma_engines)]
                    bdma_i += 1
                    eng.dma_start(
                        b_t[:, t, :width],
                        bcat3[K * BLK : (K + 1) * BLK, t, pstart : pstart + width],
                    )
                b_ts[d] = (b_t, pstart, width)

            first = K == I
            if mode == "bf16x3":
                passes = [(0, 0), (0, 1), (1, 0)]  # (A plane, B plane)
            else:
                passes = [(0, 0)]
            for pi, (ta, tb) in enumerate(passes):
                a_w = a_t[:, kb, ta, :]
                for c in range(K // 4, NB // 4):
                    d = c // 2
                    b_t, pstart, width = b_ts[d]
                    cstart = max(pstart, PHASE * c)  # global col of this MM
                    cwidth = PHASE * (c + 1) - cstart
                    o = ps[c][:, cstart - PHASE * c : PHASE]
                    boff = cstart - pstart  # offset into the b tile
                    is_first = first and pi == 0
                    is_last = pi == len(passes) - 1 and K == min(4 * c + 3, NB - 1)
                    if "nomm" not in variant:
                        nc.tensor.matmul(
                            o, a_w, b_t[:, tb, boff : boff + cwidth],
                            start=is_first, stop=is_last,
                        )

        # Evict the full output row; next I's banks free up as copies drain.
        for c in range(c0, NB // 4):
            if "nomm" in variant and "noevict" in variant:
                continue
            coff0 = max(I * BLK - PHASE * c, 0)
            w = PHASE - coff0
            ct = cpool.tile([BLK, PHASE], f32, name=f"c_{I}_{c}", tag="cst")
            nc.vector.tensor_copy(ct[:, :w], ps[c][:, coff0:PHASE])
            r0 = slot[I] * BLK
            nc.gpsimd.dma_start(
                cpart[r0 : r0 + BLK, PHASE * c + coff0 : PHASE * (c + 1)],
                ct[:, :w],
            )


def _build(mode, repeat=1, variant="full"):
    import concourse.mybir as mybir
    import concourse.tile as tile
    from concourse import bacc

    nc = bacc.Bacc(None, target_bir_lowering=False, debug=False)
    f32 = mybir.dt.float32
    nplane = 2 if mode == "bf16x3" else 1
    dt_in = {
        "bf16x3": mybir.dt.bfloat16,
        "fp32r": mybir.dt.float32r,
        "fp32": f32,
    }[mode]
    with tile.TileContext(nc) as tc:
        with (
            tc.tile_pool(name="dram", bufs=1, space="DRAM") as dram,
            tc.tile_pool(name="apool", bufs=2) as apool,
            tc.tile_pool(name="bpool", bufs=16) as bpool,
            tc.tile_pool(name="cpool", bufs=4) as cpool,
            tc.tile_pool(name="psum", bufs=1, space="PSUM") as psum_pool,
        ):
            dram_io = {
                "apack": dram.tile(
                    [BLK, NSLOT, nplane, BLK], dt_in, kind="ExternalInput",
                    name="apack", uniquify=False,
                ),
                "bcat": dram.tile(
                    [N, nplane * N], dt_in, kind="ExternalInput",
                    name="bcat", uniquify=False,
                ),
                "cpart": dram.tile(
                    [MAXB * BLK, N], f32, kind="ExternalOutput",
                    name="cpart", uniquify=False,
                ),
            }
            pid = nc.partition_id()
            pools = (apool, bpool, cpool, psum_pool)
            for c in range(N_CORES):
                with tc.If(pid == c):
                    if repeat > 1:
                        with tc.For_i(
                            0, repeat, 1, hint_engines=tuple(mybir.ALL_ENGINES)
                        ):
                            _emit_core(nc, tc, pools, dram_io, c, mode, variant)
                    else:
                        _emit_core(nc, tc, pools, dram_io, c, mode, variant)
    nc.compile()
    return nc


_cached_nc = {}


def _get_nc(mode):
    if mode not in _cached_nc:
        _cached_nc[mode] = _build(mode)
    return _cached_nc[mode]


def _host_pack(A, B, mode):
    """Build per-core apack tensors and the shared bcat tensor."""
    if mode == "bf16x3":
        import ml_dtypes

        bf16 = ml_dtypes.bfloat16
        AT = np.ascontiguousarray(A.T)
        ath = AT.astype(bf16)
        atl = (AT - ath.astype(np.float32)).astype(bf16)
        bh_ = B.astype(bf16)
        bl_ = (B - bh_.astype(np.float32)).astype(bf16)
        planes_a = [ath, atl]
        bcat = np.concatenate([bh_, bl_], axis=1)
        npdt = bf16
    else:
        AT = np.ascontiguousarray(A.T)
        planes_a = [AT]
        bcat = np.ascontiguousarray(B)
        npdt = np.float32
    nplane = len(planes_a)

    apacks = []
    for c in range(N_CORES):
        ap = np.zeros((BLK, NSLOT, nplane, BLK), dtype=npdt)
        for I in BINS[c]:
            base = ABASE[c][I]
            for j, K in enumerate(range(I, NB)):
                for t, pl in enumerate(planes_a):
                    ap[:, base + j, t, :] = pl[
                        K * BLK : (K + 1) * BLK, I * BLK : (I + 1) * BLK
                    ]
        apacks.append(ap)
    return apacks, bcat


LAST = None  # last BassKernelResults (for test harness introspection)


def kernel(A, B):
    global LAST
    import os

    from concourse.bass_utils import run_bass_kernel_spmd

    A = np.asarray(A, dtype=np.float32)
    B = np.asarray(B, dtype=np.float32)
    nc = _get_nc(MODE)
    apacks, bcat = _host_pack(A, B, MODE)
    in_maps = [{"apack": apacks[c], "bcat": bcat} for c in range(N_CORES)]
    tkw = {}
    if os.environ.get("KTRACE"):
        tkw["trace"] = True
        tkw["tmpdir"] = os.environ.get("KTRACE_DIR") or None
        tc_env = os.environ.get("KTRACE_CORES")
        if tc_env:
            tkw["trace_cores"] = [int(x) for x in tc_env.split(",")]
    res = run_bass_kernel_spmd(nc, in_maps, core_ids=list(range(N_CORES)), **tkw)
    LAST = res

    C = np.zeros((N, N), dtype=np.float32)
    for c in range(N_CORES):
        cp = res.results[c]["cpart"]
        for s, I in enumerate(sorted(BINS[c])):
            C[I * BLK : (I + 1) * BLK, I * BLK :] = cp[s * BLK : (s + 1) * BLK, I * BLK :]
    return C



# revision 6
# speedup vs baseline: 2.1806x; 2.1806x over previous
"""Triangular matmul C = triu(triu(A) @ triu(B)) on 8 TRN2 NeuronCores.

Structure: the (I, K, J) block-tetrahedron {I <= K <= J} (128x128 blocks,
N=4096 -> 32 blocks/side) is sharded by output row-block I across the 8
cores with a work-balanced assignment.  Each core runs its own statically
addressed program inside a `tc.If(partition_id == c)` block.

v2 design (vs the bf16x3 baseline at ~270us):
- Single bf16 pass (harness gate is 2e-2; measured host-sim rel err ~2e-3).
  3x less PE work, 2x less DMA than bf16x3.
- B strips are SBUF-resident: strip K = B[K*128:(K+1)*128, K*128:] is DMA'd
  ONCE per core (132KB/partition worst case) instead of once per (I,K).
- A^T blocks for all owned rows are loaded up-front (<=20KB/partition).
- Per row I: K-major sweep, full output row in up to 8 PSUM banks; bank c
  is evicted EARLY (as soon as its last contributing K = 4c+3 completes),
  overlapping eviction with the remaining K sweep.
- C is stored as bf16 (host converts to fp32; adds <=2e-3 elementwise).

The kernel takes FULL (unsharded) inputs and returns the FULL output.
"""

import numpy as np

N = 4096
BLK = 128
NB = N // BLK  # 32
N_CORES = 8
PHASE = 512  # PSUM bank width (fp32)
NBANK = 8

# Work-balanced assignment of row-blocks I to cores (work(I) = T(32-I),
# T(m)=m(m+1)/2; bins balanced to 743..752 of 5984/8=748).
BINS = [
    [0, 14, 23],
    [1, 15, 21, 25, 29],
    [2, 13, 20, 28],
    [4, 12, 16],
    [3, 10, 22],
    [6, 9, 17, 30],
    [5, 11, 19, 24, 27, 31],
    [7, 8, 18, 26],
]
MAXB = max(len(b) for b in BINS)  # output row-slots per core
# A-pack slot layout: per core, the A^T strips (one 128x128 block per slot)
# for each owned I, K = I..31, concatenated.
ABASE = [
    {I: int(np.cumsum([0] + [NB - J for J in sorted(b)])[i]) for i, I in enumerate(sorted(b))}
    for b in BINS
]
NSLOT = 80  # >= max per-core total blocks (75)
ACHUNK = 8  # A-load DMA granularity in k-blocks


def _emit_core(nc, tc, pools, dram_io, core):
    import concourse.mybir as mybir

    f32 = mybir.dt.float32
    bf16 = mybir.dt.bfloat16
    apool, bpool, cpool, psum_pool = pools
    apack, bh, cpart = dram_io["apack"], dram_io["bh"], dram_io["cpart"]

    bin_is = sorted(BINS[core])
    Imin = bin_is[0]
    slot = {I: s for s, I in enumerate(bin_is)}

    # --- A for the first row (needed immediately) ---
    a_ts = {}

    def load_a(I):
        nblk = NB - I
        base = ABASE[core][I]
        a_t = apool.tile(
            [BLK, nblk, BLK], bf16, name=f"a_{I}", tag=f"a{slot[I]}", bufs=1
        )
        for j0 in range(0, nblk, ACHUNK):
            j1 = min(j0 + ACHUNK, nblk)
            nc.gpsimd.dma_start(a_t[:, j0:j1, :], apack[:, base + j0 : base + j1, :])
        a_ts[I] = a_t

    load_a(Imin)

    # --- resident B strips, one DMA each, alternating issue engines ---
    beng = [nc.sync, nc.scalar]
    bsb = {}
    for i, K in enumerate(range(Imin, NB)):
        W = N - K * BLK
        t = bpool.tile([BLK, W], bf16, name=f"bs_{K}", tag=f"bs{K}", bufs=1)
        beng[i % 2].dma_start(t[:], bh[K * BLK : (K + 1) * BLK, K * BLK : N])
        bsb[K] = t

    # --- A for the remaining rows ---
    for I in bin_is[1:]:
        load_a(I)

    # --- compute ---
    cpeng = [
        lambda o, i: nc.vector.tensor_copy(o, i),
        lambda o, i: nc.scalar.copy(o, i),
    ]  # PSUM -> SBUF eviction copies
    steng = [nc.gpsimd, nc.sync]  # SBUF -> HBM stores
    ev = 0
    for I in bin_is:
        a_t = a_ts[I]
        c0 = I // 4
        ps = {
            c: psum_pool.tile([BLK, PHASE], f32, name=f"ps_{I}_{c}", tag=f"ps{c}")
            for c in range(c0, NBANK)
        }
        for K in range(I, NB):
            a_w = a_t[:, K - I, :]
            bt = bsb[K]
            for c in range(K // 4, NBANK):
                cstart = max(K * BLK, PHASE * c)
                cwidth = PHASE * (c + 1) - cstart
                boff = cstart - K * BLK
                o = ps[c][:, cstart - PHASE * c : PHASE]
                nc.tensor.matmul(
                    o, a_w, bt[:, boff : boff + cwidth],
                    start=(K == I), stop=(K == min(4 * c + 3, NB - 1)),
                )
            # early-evict banks whose last contributing K just ran
            for c in range(c0, NBANK):
                if min(4 * c + 3, NB - 1) == K:
                    coff0 = max(I * BLK - PHASE * c, 0)
                    w = PHASE - coff0
                    ct = cpool.tile([BLK, PHASE], bf16, name=f"c_{I}_{c}", tag="cst")
                    cpeng[ev % 2](ct[:, :w], ps[c][:, coff0:PHASE])
                    r0 = slot[I] * BLK
                    steng[ev % 2].dma_start(
                        cpart[r0 : r0 + BLK, PHASE * c + coff0 : PHASE * (c + 1)],
                        ct[:, :w],
                    )
                    ev += 1


def _build():
    import concourse.mybir as mybir
    import concourse.tile as tile
    from concourse import bacc

    nc = bacc.Bacc(None, target_bir_lowering=False, debug=False)
    f32 = mybir.dt.float32
    bf16 = mybir.dt.bfloat16
    with tile.TileContext(nc) as tc:
        with (
            tc.tile_pool(name="dram", bufs=1, space="DRAM") as dram,
            tc.tile_pool(name="apool", bufs=1) as apool,
            tc.tile_pool(name="bpool", bufs=1) as bpool,
            tc.tile_pool(name="cpool", bufs=4) as cpool,
            tc.tile_pool(name="psum", bufs=1, space="PSUM") as psum_pool,
        ):
            dram_io = {
                "apack": dram.tile(
                    [BLK, NSLOT, BLK], bf16, kind="ExternalInput",
                    name="apack", uniquify=False,
                ),
                "bh": dram.tile(
                    [N, N], bf16, kind="ExternalInput", name="bh", uniquify=False,
                ),
                "cpart": dram.tile(
                    [MAXB * BLK, N], bf16, kind="ExternalOutput",
                    name="cpart", uniquify=False,
                ),
            }
            pid = nc.partition_id()
            pools = (apool, bpool, cpool, psum_pool)
            for c in range(N_CORES):
                with tc.If(pid == c):
                    _emit_core(nc, tc, pools, dram_io, c)
    nc.compile()
    return nc


_cached_nc = None


def _get_nc():
    global _cached_nc
    if _cached_nc is None:
        _cached_nc = _build()
    return _cached_nc


def _host_pack(A, B):
    """Build per-core apack tensors (A^T blocks, bf16) and bh (B, bf16)."""
    import ml_dtypes

    bf16 = ml_dtypes.bfloat16
    ath = np.ascontiguousarray(A.T).astype(bf16)
    bh = np.ascontiguousarray(B.astype(bf16))

    apacks = []
    for c in range(N_CORES):
        ap = np.zeros((BLK, NSLOT, BLK), dtype=bf16)
        for I in BINS[c]:
            base = ABASE[c][I]
            for j, K in enumerate(range(I, NB)):
                ap[:, base + j, :] = ath[
                    K * BLK : (K + 1) * BLK, I * BLK : (I + 1) * BLK
                ]
        apacks.append(ap)
    return apacks, bh


LAST = None  # last BassKernelResults (for test harness introspection)


def kernel(A, B):
    global LAST
    import os

    from concourse.bass_utils import run_bass_kernel_spmd

    A = np.asarray(A, dtype=np.float32)
    B = np.asarray(B, dtype=np.float32)
    nc = _get_nc()
    apacks, bh = _host_pack(A, B)
    in_maps = [{"apack": apacks[c], "bh": bh} for c in range(N_CORES)]
    tkw = {}
    if os.environ.get("KTRACE"):
        tkw["trace"] = True
        tkw["tmpdir"] = os.environ.get("KTRACE_DIR") or None
        tc_env = os.environ.get("KTRACE_CORES")
        if tc_env:
            tkw["trace_cores"] = [int(x) for x in tc_env.split(",")]
    res = run_bass_kernel_spmd(nc, in_maps, core_ids=list(range(N_CORES)), **tkw)
    LAST = res

    C = np.zeros((N, N), dtype=np.float32)
    for c in range(N_CORES):
        cp = res.results[c]["cpart"]
        for s, I in enumerate(sorted(BINS[c])):
            C[I * BLK : (I + 1) * BLK, I * BLK :] = cp[
                s * BLK : (s + 1) * BLK, I * BLK :
            ].astype(np.float32)
    return C


# revision 7
# speedup vs baseline: 2.6735x; 1.2260x over previous
"""Triangular matmul C = triu(triu(A) @ triu(B)) on 8 TRN2 NeuronCores.

Structure: the (I, K, J) block-tetrahedron {I <= K <= J} (128x128 blocks,
N=4096 -> 32 blocks/side) is sharded by output row-block I across the 8
cores with a work-balanced assignment.  Each core runs its own statically
addressed program inside a `tc.If(partition_id == c)` block.

v2 design (vs the bf16x3 baseline at ~270us):
- Single bf16 pass (harness gate is 2e-2; measured host-sim rel err ~2e-3).
  3x less PE work, 2x less DMA than bf16x3.
- B strips are SBUF-resident: strip K = B[K*128:(K+1)*128, K*128:] is DMA'd
  ONCE per core (132KB/partition worst case) instead of once per (I,K).
- A^T blocks for all owned rows are loaded up-front (<=20KB/partition).
- Per row I: K-major sweep, full output row in up to 8 PSUM banks; bank c
  is evicted EARLY (as soon as its last contributing K = 4c+3 completes),
  overlapping eviction with the remaining K sweep.
- C is stored as bf16 (host converts to fp32; adds <=2e-3 elementwise).

The kernel takes FULL (unsharded) inputs and returns the FULL output.
"""

import numpy as np

N = 4096
BLK = 128
NB = N // BLK  # 32
N_CORES = 8
PHASE = 512  # PSUM bank width (fp32)
NBANK = 8

# Work-balanced assignment of row-blocks I to cores (work(I) = T(32-I),
# T(m)=m(m+1)/2; bins balanced to 743..752 of 5984/8=748).
BINS = [
    [0, 14, 23],
    [1, 15, 21, 25, 29],
    [2, 13, 20, 28],
    [4, 12, 16],
    [3, 10, 22],
    [6, 9, 17, 30],
    [5, 11, 19, 24, 27, 31],
    [7, 8, 18, 26],
]
MAXB = max(len(b) for b in BINS)  # output row-slots per core
# A-pack slot layout: per core, the A^T strips (one 128x128 block per slot)
# for each owned I, K = I..31, concatenated.
ABASE = [
    {I: int(np.cumsum([0] + [NB - J for J in sorted(b)])[i]) for i, I in enumerate(sorted(b))}
    for b in BINS
]
NSLOT = 80  # >= max per-core total blocks (75)
ACHUNK = 8  # A-load DMA granularity in k-blocks


def _emit_core(nc, tc, pools, dram_io, core):
    import concourse.mybir as mybir

    f32 = mybir.dt.float32
    bf16 = mybir.dt.bfloat16
    apool, bpool, cpool, psum_pool = pools
    apack, bh, cpart = dram_io["apack"], dram_io["bh"], dram_io["cpart"]

    bin_is = sorted(BINS[core])
    Imin = bin_is[0]
    slot = {I: s for s, I in enumerate(bin_is)}

    # --- A for the first row (needed immediately) ---
    a_ts = {}

    def load_a(I):
        nblk = NB - I
        base = ABASE[core][I]
        a_t = apool.tile(
            [BLK, nblk, BLK], bf16, name=f"a_{I}", tag=f"a{slot[I]}", bufs=1
        )
        for j0 in range(0, nblk, ACHUNK):
            j1 = min(j0 + ACHUNK, nblk)
            nc.gpsimd.dma_start(a_t[:, j0:j1, :], apack[:, base + j0 : base + j1, :])
        a_ts[I] = a_t

    load_a(Imin)

    # --- resident B strips, one DMA each, alternating issue engines ---
    beng = [nc.sync, nc.scalar]
    bsb = {}
    for i, K in enumerate(range(Imin, NB)):
        W = N - K * BLK
        t = bpool.tile([BLK, W], bf16, name=f"bs_{K}", tag=f"bs{K}", bufs=1)
        beng[i % 2].dma_start(t[:], bh[K * BLK : (K + 1) * BLK, K * BLK : N])
        bsb[K] = t

    # --- A for the remaining rows ---
    for I in bin_is[1:]:
        load_a(I)

    # --- compute ---
    cpeng = [
        lambda o, i: nc.vector.tensor_copy(o, i),
        lambda o, i: nc.scalar.copy(o, i),
    ]  # PSUM -> SBUF eviction copies
    steng = [nc.gpsimd, nc.sync]  # SBUF -> HBM stores
    ev = 0
    for I in bin_is:
        a_t = a_ts[I]
        c0 = I // 4
        ps = {
            c: psum_pool.tile([BLK, PHASE], f32, name=f"ps_{I}_{c}", tag=f"ps{c}")
            for c in range(c0, NBANK)
        }
        for K in range(I, NB):
            a_w = a_t[:, K - I, :]
            bt = bsb[K]
            for c in range(K // 4, NBANK):
                cstart = max(K * BLK, PHASE * c)
                cwidth = PHASE * (c + 1) - cstart
                boff = cstart - K * BLK
                o = ps[c][:, cstart - PHASE * c : PHASE]
                nc.tensor.matmul(
                    o, a_w, bt[:, boff : boff + cwidth],
                    start=(K == I), stop=(K == min(4 * c + 3, NB - 1)),
                )
            # early-evict banks whose last contributing K just ran
            for c in range(c0, NBANK):
                if min(4 * c + 3, NB - 1) == K:
                    coff0 = max(I * BLK - PHASE * c, 0)
                    w = PHASE - coff0
                    ct = cpool.tile([BLK, PHASE], bf16, name=f"c_{I}_{c}", tag="cst")
                    cpeng[ev % 2](ct[:, :w], ps[c][:, coff0:PHASE])
                    r0 = slot[I] * BLK
                    steng[ev % 2].dma_start(
                        cpart[r0 : r0 + BLK, PHASE * c + coff0 : PHASE * (c + 1)],
                        ct[:, :w],
                    )
                    ev += 1


def _build():
    import concourse.mybir as mybir
    import concourse.tile as tile
    from concourse import bacc

    nc = bacc.Bacc(None, target_bir_lowering=False, debug=False)
    f32 = mybir.dt.float32
    bf16 = mybir.dt.bfloat16
    with tile.TileContext(nc) as tc:
        with (
            tc.tile_pool(name="dram", bufs=1, space="DRAM") as dram,
            tc.tile_pool(name="apool", bufs=1) as apool,
            tc.tile_pool(name="bpool", bufs=1) as bpool,
            tc.tile_pool(name="cpool", bufs=4) as cpool,
            tc.tile_pool(name="psum", bufs=1, space="PSUM") as psum_pool,
        ):
            dram_io = {
                "apack": dram.tile(
                    [BLK, NSLOT, BLK], bf16, kind="ExternalInput",
                    name="apack", uniquify=False,
                ),
                "bh": dram.tile(
                    [N, N], bf16, kind="ExternalInput", name="bh", uniquify=False,
                ),
                "cpart": dram.tile(
                    [MAXB * BLK, N], bf16, kind="ExternalOutput",
                    name="cpart", uniquify=False,
                ),
            }
            pid = nc.partition_id()
            pools = (apool, bpool, cpool, psum_pool)
            for c in tc.Switch(pid, N_CORES):
                _emit_core(nc, tc, pools, dram_io, c)
    nc.compile()
    return nc


_cached_nc = None


def _get_nc():
    global _cached_nc
    if _cached_nc is None:
        _cached_nc = _build()
    return _cached_nc


def _host_pack(A, B):
    """Build per-core apack tensors (A^T blocks, bf16) and bh (B, bf16)."""
    import ml_dtypes

    bf16 = ml_dtypes.bfloat16
    ath = np.ascontiguousarray(A.T).astype(bf16)
    bh = np.ascontiguousarray(B.astype(bf16))

    apacks = []
    for c in range(N_CORES):
        ap = np.zeros((BLK, NSLOT, BLK), dtype=bf16)
        for I in BINS[c]:
            base = ABASE[c][I]
            for j, K in enumerate(range(I, NB)):
                ap[:, base + j, :] = ath[
                    K * BLK : (K + 1) * BLK, I * BLK : (I + 1) * BLK
                ]
        apacks.append(ap)
    return apacks, bh


LAST = None  # last BassKernelResults (for test harness introspection)


def kernel(A, B):
    global LAST
    import os

    from concourse.bass_utils import run_bass_kernel_spmd

    A = np.asarray(A, dtype=np.float32)
    B = np.asarray(B, dtype=np.float32)
    nc = _get_nc()
    apacks, bh = _host_pack(A, B)
    in_maps = [{"apack": apacks[c], "bh": bh} for c in range(N_CORES)]
    tkw = {}
    if os.environ.get("KTRACE"):
        tkw["trace"] = True
        tkw["tmpdir"] = os.environ.get("KTRACE_DIR") or None
        tc_env = os.environ.get("KTRACE_CORES")
        if tc_env:
            tkw["trace_cores"] = [int(x) for x in tc_env.split(",")]
    res = run_bass_kernel_spmd(nc, in_maps, core_ids=list(range(N_CORES)), **tkw)
    LAST = res

    C = np.zeros((N, N), dtype=np.float32)
    for c in range(N_CORES):
        cp = res.results[c]["cpart"]
        for s, I in enumerate(sorted(BINS[c])):
            C[I * BLK : (I + 1) * BLK, I * BLK :] = cp[
                s * BLK : (s + 1) * BLK, I * BLK :
            ].astype(np.float32)
    return C
